# revision 2
# baseline (speedup 1.0000x reference)
"""KNN WRMF negative sampler on 8 Trainium2 NeuronCores.

Per 128-row tile, per sample k (32 of them), ONE fused DVE instruction:

    scan(ADD, (cum[n] < u_k) * dpn[s, n])   over the [2, 99] stream
                                            s=0: prob diffs, s=1: knn diffs

u_k rides the per-partition scalar slot (C0), so there is no separate
compare/mask pass on any engine.  Because each instruction restarts its
scan at zero:
    prob[k] = prob_tab[.,0] + sc[k, 0, 98]
    neg[k]  = knnf[.,0]     + (sc[k, 1, 98] - sc[k, 0, 98])
The knn-diff prefix inside one instruction is nb[j]-nb[0] (bounded by
1e5), so the float error of the mixed prob+knn prefix stays << 0.5 and
the final round-to-int recovers the exact neighbour id.

Per-row tables come from ONE indirect row gather of a host-built combo
table [NLOC+1, 300]: [cum(100), dprob(99), dknn(99), prob0, knnf0],
pre-shifted so knn's -1 indexing and int->float casts happen on host.
"""

import numpy as np
from contextlib import ExitStack

import concourse.bass as bass
import concourse.bacc as bacc
import concourse.mybir as mybir
import concourse.tile as tile
import concourse.dve_ops as dve_ops
from concourse.dve_ops import DveOp
from concourse.dve_spec import (
    Spec, lower, scan, Src0, Src1, C0, AluOp, _has_src1,
)
from concourse.dve_uop import DveOpSpec
from concourse.bass_utils import run_bass_kernel_spmd

P = 128          # partitions
T = 4            # row-tiles per core
RPC = P * T      # rows per core
K = 32           # samples per row
N = 100          # neighbours per row
NCORES = 8
NLOC = 100000
CW = 3 * N       # combo row: 100 cum + 99 dp + 99 dn + pt0 + nb0

_cache = {}


def _register_op(name, spec, subdim=False):
    if name in dve_ops._SUB_OPCODE_FOR_NAME:
        return next(o for o in dve_ops.OPS if o.name == name)
    row = max(dve_ops._SUB_OPCODE_FOR_NAME.values()) + 1
    assert row < 0x20
    shas = {}
    for ver in ("v3", "v4"):
        uops = lower(spec, ver=ver)
        shas[ver] = DveOpSpec(
            name=name, opcode=row, uops=uops, rd1_en=_has_src1(spec)
        ).sha(ver)
    op = DveOp(name, spec, subdim=subdim, uops_sha=shas)
    dve_ops.OPS.append(op)
    dve_ops.CUSTOM_DVE_SPECS[name] = spec
    dve_ops._SUB_OPCODE_FOR_NAME[name] = row
    return op


def _register_cmp_scan():
    # out = cumsum over the free stream of 1[Src0 < c0] * Src1
    def _ref(in0, in1, c0, c1, c2):
        Pp = in0.shape[0]
        x = (in0.reshape(Pp, -1) < c0.reshape(Pp, 1)).astype(np.float32) \
            * in1.reshape(Pp, -1)
        return np.cumsum(x, axis=1).reshape(in0.shape).astype(np.float32)

    spec = Spec(body=scan(AluOp.ADD, (Src0 < C0) * Src1), reference=_ref)
    return _register_op("CMP_SCALAR_MUL_SCAN_ANT1", spec)


def _build():
    if "nc" in _cache:
        return _cache["nc"]
    cmp_scan = _register_cmp_scan()
    nc = bacc.Bacc("TRN2")
    f32, i32 = mybir.dt.float32, mybir.dt.int32
    loc_d = nc.dram_tensor("loc", [RPC, 1], i32, kind="ExternalInput").ap()
    uni = nc.dram_tensor("uni", [RPC, K], f32, kind="ExternalInput").ap()
    combo = nc.dram_tensor("combo", [NLOC + 1, CW], f32, kind="ExternalInput").ap()
    oneg = nc.dram_tensor("oneg", [RPC, K], i32, kind="ExternalOutput").ap()
    oprob = nc.dram_tensor("oprob", [RPC, K], f32, kind="ExternalOutput").ap()

    SUB = mybir.AluOpType.subtract
    ADD = mybir.AluOpType.add
    oneg_r = oneg.rearrange("(t p) k -> p t k", p=P)
    oprob_r = oprob.rearrange("(t p) k -> p t k", p=P)

    with tile.TileContext(nc) as tc, ExitStack() as ctx:
        pool = ctx.enter_context(tc.tile_pool(name="m", bufs=1))
        big = ctx.enter_context(tc.tile_pool(name="big", bufs=2))
        cbp = ctx.enter_context(tc.tile_pool(name="cbp", bufs=4))

        loc = pool.tile([P, T], i32)
        nc.sync.dma_start(loc[:], loc_d.rearrange("(t p) c -> p (t c)", p=P))
        ut = pool.tile([P, T, K], f32)
        nc.sync.dma_start(ut[:], uni.rearrange("(t p) k -> p t k", p=P))

        for t in range(T):
            cb = cbp.tile([P, CW], f32, tag="cb")
            nc.gpsimd.indirect_dma_start(
                out=cb[:], out_offset=None, in_=combo[:],
                in_offset=bass.IndirectOffsetOnAxis(ap=loc[:, t:t + 1], axis=0))
            c99 = cb[:, 0:N - 1]                    # cum[0:99]
            dpn = cb[:, N:N + 2 * (N - 1)]          # [dp(99) | dn(99)]
            pt0 = cb[:, 3 * N - 2:3 * N - 1]
            nb0 = cb[:, 3 * N - 1:3 * N]

            sc = big.tile([P, K, 2, N - 1], f32, tag="sc")
            for kk in range(K):
                nc.vector._custom_dve(
                    cmp_scan,
                    out=sc[:, kk, :, :],
                    in0=c99[:, None, :].to_broadcast([P, 2, N - 1]),
                    in1=dpn.rearrange("p (s n) -> p s n", s=2),
                    s0=ut[:, t, kk:kk + 1],
                )

            # segment ends: e0 = prob take-along - pt0, e1 - e0 = knn - nb0
            e0 = big.tile([P, K], f32, tag="e0")
            e1 = big.tile([P, K], f32, tag="e1")
            nc.scalar.copy(e0[:], sc[:, :, 0, N - 2])
            nc.scalar.copy(e1[:], sc[:, :, 1, N - 2])

            oprob_out = big.tile([P, K], f32, tag="opb")
            nc.vector.tensor_scalar_add(oprob_out[:], e0[:], pt0)
            onf = big.tile([P, K], f32, tag="onf")
            nc.vector.scalar_tensor_tensor(
                out=onf[:], in0=e1[:], scalar=nb0, in1=e0[:],
                op0=ADD, op1=SUB)
            oneg_out = big.tile([P, K], i32, tag="onb")
            nc.vector.tensor_copy(oneg_out[:], onf[:])
            nc.sync.dma_start(oprob_r[:, t, :], oprob_out[:])
            nc.sync.dma_start(oneg_r[:, t, :], oneg_out[:])
    nc.compile()
    _cache["nc"] = nc
    return nc


def _make_combo(probs_table, cum_probs_table, knn_results):
    pt = np.asarray(probs_table, dtype=np.float32)
    ct = np.asarray(cum_probs_table, dtype=np.float32)
    kt = np.asarray(knn_results)
    combo = np.empty((NLOC + 1, CW), dtype=np.float32)
    combo[:, 0:N] = ct
    combo[:, N:2 * N - 1] = pt[:, 1:] - pt[:, :-1]
    knf = kt.astype(np.float32)
    combo[1:, 2 * N - 1:3 * N - 2] = knf[:, 1:] - knf[:, :-1]
    combo[0, 2 * N - 1:3 * N - 2] = 0.0
    combo[:, 3 * N - 2] = pt[:, 0]
    combo[1:, 3 * N - 1] = knf[:, 0]
    combo[0, 3 * N - 1] = 0.0
    return np.ascontiguousarray(combo)


def kernel(trg_seq, k, user, uniforms, knn_results, probs_table, cum_probs_table,
           **_ignored):
    loc = np.ascontiguousarray(np.asarray(trg_seq)[:, 1:2].astype(np.int32))
    uniforms = np.ascontiguousarray(np.asarray(uniforms, dtype=np.float32))
    combo = _make_combo(probs_table, cum_probs_table, knn_results)

    nc = _build()
    in_maps = []
    for c in range(NCORES):
        sl = slice(c * RPC, (c + 1) * RPC)
        in_maps.append({
            "loc": loc[sl],
            "uni": uniforms[sl],
            "combo": combo,
        })
    res = run_bass_kernel_spmd(nc, in_maps, core_ids=list(range(NCORES)))
    neg = np.concatenate([res.results[c]["oneg"] for c in range(NCORES)], axis=0)
    prob = np.concatenate([res.results[c]["oprob"] for c in range(NCORES)], axis=0)
    return neg, prob


# revision 3
# speedup vs baseline: 1.0087x; 1.0087x over previous
"""KNN WRMF negative sampler on 8 Trainium2 NeuronCores.

Like v3 (per-k fused compare+mul+scan DVE instructions over the
contiguous [dprob|dknn] pages), plus:
 - stride-0 output APs: each scan writes its running prefix into a
   single slot per page, so the LAST write is the segment end — the
   [P,K,2,99] scratch and the ACT extract pass disappear; the scan
   output IS the [P,K,2] ends tile.
 - outputs computed and stored per k-half, so the final tile's
   deprefix/cast/store chain hides under the second half's scans.
"""

import numpy as np
from contextlib import ExitStack

import concourse.bass as bass
import concourse.bacc as bacc
import concourse.mybir as mybir
import concourse.tile as tile
import concourse.dve_ops as dve_ops
from concourse.dve_ops import DveOp
from concourse.dve_spec import (
    Spec, lower, scan, Src0, Src1, C0, AluOp, _has_src1,
)
from concourse.dve_uop import DveOpSpec
from concourse.bass_utils import run_bass_kernel_spmd

P = 128
T = 4
RPC = P * T
K = 32
H = K // 2
N = 100
NCORES = 8
NLOC = 100000
CW = 3 * N

SMALLS_ON_POOL = True

_cache = {}


def _register_op(name, spec, subdim=False):
    if name in dve_ops._SUB_OPCODE_FOR_NAME:
        return next(o for o in dve_ops.OPS if o.name == name)
    row = max(dve_ops._SUB_OPCODE_FOR_NAME.values()) + 1
    assert row < 0x20
    shas = {}
    for ver in ("v3", "v4"):
        uops = lower(spec, ver=ver)
        shas[ver] = DveOpSpec(
            name=name, opcode=row, uops=uops, rd1_en=_has_src1(spec)
        ).sha(ver)
    op = DveOp(name, spec, subdim=subdim, uops_sha=shas)
    dve_ops.OPS.append(op)
    dve_ops.CUSTOM_DVE_SPECS[name] = spec
    dve_ops._SUB_OPCODE_FOR_NAME[name] = row
    return op


def _register_cmp_scan():
    def _ref(in0, in1, c0, c1, c2):
        Pp = in0.shape[0]
        x = (in0.reshape(Pp, -1) < c0.reshape(Pp, 1)).astype(np.float32) \
            * in1.reshape(Pp, -1)
        return np.cumsum(x, axis=1).reshape(in0.shape).astype(np.float32)

    spec = Spec(body=scan(AluOp.ADD, (Src0 < C0) * Src1), reference=_ref)
    return _register_op("CMP_SCALAR_MUL_SCAN_ANT1", spec)


def _build():
    if "nc" in _cache:
        return _cache["nc"]
    cmp_scan = _register_cmp_scan()
    nc = bacc.Bacc("TRN2")
    f32, i32 = mybir.dt.float32, mybir.dt.int32
    loc_d = nc.dram_tensor("loc", [RPC, 1], i32, kind="ExternalInput").ap()
    uni = nc.dram_tensor("uni", [RPC, K], f32, kind="ExternalInput").ap()
    combo = nc.dram_tensor("combo", [NLOC + 1, CW], f32, kind="ExternalInput").ap()
    oneg = nc.dram_tensor("oneg", [RPC, K], i32, kind="ExternalOutput").ap()
    oprob = nc.dram_tensor("oprob", [RPC, K], f32, kind="ExternalOutput").ap()

    SUB = mybir.AluOpType.subtract
    ADD = mybir.AluOpType.add
    oneg_r = oneg.rearrange("(t p) k -> p t k", p=P)
    oprob_r = oprob.rearrange("(t p) k -> p t k", p=P)
    eng = nc.gpsimd if SMALLS_ON_POOL else nc.vector

    with tile.TileContext(nc) as tc, ExitStack() as ctx:
        pool = ctx.enter_context(tc.tile_pool(name="m", bufs=1))
        big = ctx.enter_context(tc.tile_pool(name="big", bufs=3))
        cbp = ctx.enter_context(tc.tile_pool(name="cbp", bufs=4))

        loc = pool.tile([P, T], i32)
        nc.sync.dma_start(loc[:], loc_d.rearrange("(t p) c -> p (t c)", p=P))
        ut = pool.tile([P, T, K], f32)
        nc.sync.dma_start(ut[:], uni.rearrange("(t p) k -> p t k", p=P))

        for t in range(T):
            cb = cbp.tile([P, CW], f32, tag="cb")
            nc.gpsimd.indirect_dma_start(
                out=cb[:], out_offset=None, in_=combo[:],
                in_offset=bass.IndirectOffsetOnAxis(ap=loc[:, t:t + 1], axis=0))
            c99 = cb[:, 0:N - 1]
            dpn = cb[:, N:N + 2 * (N - 1)]
            pt0 = cb[:, 3 * N - 2:3 * N - 1]
            nb0 = cb[:, 3 * N - 1:3 * N]

            ends = big.tile([P, K, 2], f32, tag="ends")
            for h in range(2):
                for kk in range(h * H, (h + 1) * H):
                    nc.vector._custom_dve(
                        cmp_scan,
                        out=ends[:, kk, :][:, :, None].to_broadcast(
                            [P, 2, N - 1]),
                        in0=c99[:, None, :].to_broadcast([P, 2, N - 1]),
                        in1=dpn.rearrange("p (s n) -> p s n", s=2),
                        s0=ut[:, t, kk:kk + 1],
                    )
                ks = slice(h * H, (h + 1) * H)
                e0 = ends[:, ks, 0]
                e1 = ends[:, ks, 1]
                opb = big.tile([P, H], f32, tag=f"opb{h}")
                onf = big.tile([P, H], f32, tag=f"onf{h}")
                onb = big.tile([P, H], i32, tag=f"onb{h}")
                if SMALLS_ON_POOL:
                    # Pool only supports add/sub/mult TT ops
                    nc.gpsimd.tensor_tensor(
                        out=opb[:], in0=e0,
                        in1=pt0.to_broadcast([P, H]), op=ADD)
                    tmpn = big.tile([P, H], f32, tag=f"tmpn{h}")
                    nc.gpsimd.tensor_tensor(
                        out=tmpn[:], in0=e1,
                        in1=nb0.to_broadcast([P, H]), op=ADD)
                    nc.gpsimd.tensor_tensor(
                        out=onf[:], in0=tmpn[:], in1=e0, op=SUB)
                else:
                    nc.vector.tensor_scalar_add(opb[:], e0, pt0)
                    nc.vector.scalar_tensor_tensor(
                        out=onf[:], in0=e1, scalar=nb0, in1=e0,
                        op0=ADD, op1=SUB)
                nc.vector.tensor_copy(onb[:], onf[:])
                nc.sync.dma_start(oprob_r[:, t, ks], opb[:])
                nc.sync.dma_start(oneg_r[:, t, ks], onb[:])
    nc.compile()
    _cache["nc"] = nc
    return nc


def _make_combo(probs_table, cum_probs_table, knn_results):
    pt = np.asarray(probs_table, dtype=np.float32)
    ct = np.asarray(cum_probs_table, dtype=np.float32)
    kt = np.asarray(knn_results)
    combo = np.empty((NLOC + 1, CW), dtype=np.float32)
    combo[:, 0:N] = ct
    combo[:, N:2 * N - 1] = pt[:, 1:] - pt[:, :-1]
    knf = kt.astype(np.float32)
    combo[1:, 2 * N - 1:3 * N - 2] = knf[:, 1:] - knf[:, :-1]
    combo[0, 2 * N - 1:3 * N - 2] = 0.0
    combo[:, 3 * N - 2] = pt[:, 0]
    combo[1:, 3 * N - 1] = knf[:, 0]
    combo[0, 3 * N - 1] = 0.0
    return np.ascontiguousarray(combo)


def kernel(trg_seq, k, user, uniforms, knn_results, probs_table, cum_probs_table,
           **_ignored):
    loc = np.ascontiguousarray(np.asarray(trg_seq)[:, 1:2].astype(np.int32))
    uniforms = np.ascontiguousarray(np.asarray(uniforms, dtype=np.float32))
    combo = _make_combo(probs_table, cum_probs_table, knn_results)

    nc = _build()
    in_maps = []
    for c in range(NCORES):
        sl = slice(c * RPC, (c + 1) * RPC)
        in_maps.append({
            "loc": loc[sl],
            "uni": uniforms[sl],
            "combo": combo,
        })
    res = run_bass_kernel_spmd(nc, in_maps, core_ids=list(range(NCORES)))
    neg = np.concatenate([res.results[c]["oneg"] for c in range(NCORES)], axis=0)
    prob = np.concatenate([res.results[c]["oprob"] for c in range(NCORES)], axis=0)
    return neg, prob
